# revision 38
# baseline (speedup 1.0000x reference)
"""Trainium2 Bass kernel for a Chemprop GNN message-passing layer.

Reference computation (single layer, n_nodes=50000, n_edges=300000, hidden=256):
    H   = relu(E)                                  # [E, 256]
    M_v = segment_sum(H, dest, n_nodes)            # [V, 256]
    out = (M_v[src] - H[rev]) @ W.T + b            # [E, 256]

Distribution over 8 NeuronCores (zero collectives):
  * relu is PRE-APPLIED on the host (both the permuted phase-1 stream and
    the per-core E replica hold relu(E)), so no on-device activations.
  * Nodes are RELABELED on the host: a greedy balancer assigns each node a
    (core, block, lane) so every one of the 400 (core, block) buckets of
    <=128 node lanes sees ~equal dest-edge and src-edge counts. This gets
    CPB1=CPB2=6 chunks of 128 edge slots per block (vs 7 with the naive
    contiguous node split) - a ~14% cut in every per-chunk cost.
  * Phase 1 (segment sum): edges are grouped by dest-node bucket on the
    host; each core receives its edges' relu'd E-rows PRE-PERMUTED into
    (block, chunk, lane) slot order. The device streams them contiguously
    and accumulates per 128-node block with one-hot selection matmuls:
    M_v_block += S_chunk.T @ H_chunk where S[e, n] = (dest_lane[e] == n),
    built on-device via is_equal vs an iota row. M_v lives entirely in
    SBUF (50 blocks x [128, 256] fp16).
  * Phase 2 (gather-subtract-linear): edges are grouped by src-node bucket,
    so M_v[src] expansion is a local one-hot matmul Pv = R.T @ Mv_block
    with R[n, e] = (src_lane[e] == n). Only the reverse-edge term needs
    indirect gathers: relu'd E[rev] rows are fetched
    128-rows-per-instruction from the per-core replica. M_uv = Pv - E_r[rev]
    is transposed on the PE (two 128x128 transposes) and multiplied by W.T
    via two accumulating matmuls; bias is fused into the PSUM->SBUF copy.
    Output rows are written fp16 (halves the output DMA) contiguously in
    slot order; the host scatters them back to original edge order as f32.
"""

import sys
from contextlib import ExitStack

import numpy as np

sys.path.insert(0, "/opt/trn_rl_repo")

import concourse.bass as bass
import concourse.bacc as bacc
import concourse.tile as tile
from concourse import mybir
from concourse.bass_utils import run_bass_kernel_spmd

import ml_dtypes

MM_DT = "f16"  # "f32" | "bf16" | "f16" — dtype of the matmul path.
OH_DT = "f8"   # host-built one-hot dtype: "f8" (half the stream) | "f16"
GB = 1         # rev-gather chunks per indirect DMA (6-chunk broke on HW)
# f16 measured: rel err 4.6e-4, ~410 us/iter; f32: rel err 1.5e-7, ~720 us.
# timing-only ablation switches (break correctness when nonzero)
SKIP_P1 = False      # skip phase-1 segment sum
SKIP_REV = False     # skip rev indirect gathers
SKIP_LIN = False     # skip transpose+linear (write muv directly)
SB_BUFS = 4          # sbuf working-pool depth
PS_BUFS = (2, 2, 2, 2)  # psum bufs: mv, pv, tr, out (sum of banks <= 8)
TCOPY_ACT = False    # PSUM->SBUF transpose copy on ScalarE instead of DVE
BIAS_PE = False      # measured worse on HW (ACT copy slow); keep DVE bias-add

N_NODES = 50000
N_EDGES = 300000
HID = 256
NC = 8
P = 128
NBLK = 50                    # node blocks per core (relabeled buckets)
NBKT = NC * NBLK             # 400 buckets of <=128 nodes
PAD_LANE = 200.0             # sentinel lane value -> one-hot row of zeros


def _relabel(dest, src):
    """Assign each node a (bucket, lane) so that per-bucket dest-edge and
    src-edge loads are balanced (target 768 = 6 chunks of 128 each)."""
    dd = np.bincount(dest, minlength=N_NODES).astype(np.int64)
    sd = np.bincount(src, minlength=N_NODES).astype(np.int64)
    order = np.argsort(-(dd + sd), kind="stable")
    loads_d = np.zeros(NBKT, np.int64)
    loads_s = np.zeros(NBKT, np.int64)
    counts = np.zeros(NBKT, np.int64)
    bucket = np.empty(N_NODES, np.int64)
    lane = np.empty(N_NODES, np.int64)
    for n in order:
        cost = np.maximum(loads_d + dd[n], loads_s + sd[n]).astype(np.float64)
        cost[counts >= P] = np.inf
        j = int(np.argmin(cost))
        bucket[n] = j
        lane[n] = counts[j]
        counts[j] += 1
        loads_d[j] += dd[n]
        loads_s[j] += sd[n]
    cpb1 = int(-(-loads_d.max() // P))
    cpb2 = int(-(-loads_s.max() // P))
    return bucket, lane, cpb1, cpb2


def _group_slots(node_ids, bucket, lane):
    """Group edges by the bucket of node_ids[e]; assign (chunk, lane) slots.

    Returns (order, core, blk, j, p, lane): arrays over edges in grouped
    order; edge order[i] sits at core[i], block blk[i], chunk j[i], lane p[i],
    and selects node lane lane[i] within the block.
    """
    g = bucket[node_ids]
    order = np.argsort(g, kind="stable")
    gs = g[order]
    starts = np.searchsorted(gs, np.arange(NBKT))
    rank = np.arange(node_ids.shape[0]) - starts[gs]
    j = rank >> 7
    p = rank & 127
    return order, gs // NBLK, gs % NBLK, j, p, lane[node_ids][order]


def prepare(E, edge_index, rev_index, W, b):
    """Host-side sharding. Returns (in_maps, meta)."""
    E = np.maximum(np.ascontiguousarray(E, dtype=np.float32), 0.0)  # relu
    src = np.asarray(edge_index[0], dtype=np.int64)
    dest = np.asarray(edge_index[1], dtype=np.int64)
    rev = np.asarray(rev_index, dtype=np.int64)
    W = np.asarray(W, dtype=np.float32)
    b = np.asarray(b, dtype=np.float32)

    bucket, lane_map, CPB1, CPB2 = _relabel(dest, src)

    # ---- phase 1: dest-grouped permuted sharding of E ----
    o1, c1, blk1, j1, p1, lane1 = _group_slots(dest, bucket, lane_map)
    R1 = NBLK * CPB1 * P  # rows per core in E_p1
    row1 = blk1 * (CPB1 * P) + j1 * P + p1
    col1 = blk1 * CPB1 + j1

    # ---- phase 2: src-grouped slots ----
    o2, c2, blk2, j2, p2, lane2 = _group_slots(src, bucket, lane_map)
    R2 = NBLK * CPB2 * P
    row2 = blk2 * (CPB2 * P) + j2 * P + p2
    col2 = blk2 * CPB2 + j2

    npmm = {"f32": np.float32, "bf16": ml_dtypes.bfloat16,
        "f16": np.float16}[MM_DT]
    npoh = {"f8": ml_dtypes.float8_e4m3, "f16": np.float16}[OH_DT]
    E_rep = E if MM_DT == "f32" else E.astype(npmm)
    Wt_stack = np.ascontiguousarray(W.T.reshape(2, P, HID)).astype(npmm)
    bias_tile = np.ascontiguousarray(np.broadcast_to(b, (P, HID)))
    identity = np.eye(P, dtype=np.float32).astype(npmm)

    in_maps = []
    metas = []
    for c in range(NC):
        m1 = c1 == c
        e1 = o1[m1]
        E_p1 = np.zeros((R1, HID), npmm)
        E_p1[row1[m1]] = E[e1].astype(npmm)
        # host-built one-hot S: lhsT of the phase-1 matmul, partition = edge
        # lane (contraction with h rows), free = (chunk, node lane)
        S = np.zeros((P, NBLK * CPB1 * P), npoh)
        S[p1[m1], col1[m1] * P + lane1[m1]] = 1.0

        m2 = c2 == c
        e2 = o2[m2]
        rev_ids = np.zeros((P, NBLK * CPB2), np.int32)
        rev_ids[p2[m2], col2[m2]] = rev[e2].astype(np.int32)
        # host-built one-hot R: lhsT of the expand matmul, partition = node
        # lane (contraction with mv rows), free = (chunk, edge lane)
        Rh = np.zeros((P, NBLK * CPB2 * P), npoh)
        Rh[lane2[m2], col2[m2] * P + p2[m2]] = 1.0

        in_maps.append({
            "E_full": E_rep,
            "E_p1": E_p1,
            "S": S,
            "rev_ids": rev_ids,
            "R": Rh,
            "Wt": Wt_stack,
            "bias": bias_tile,
            "ident": identity,
        })
        metas.append({"e2": e2, "row2": row2[m2]})

    meta = {"CPB1": CPB1, "CPB2": CPB2, "metas": metas}
    return in_maps, meta


def build_program(CPB1, CPB2, reps=1):
    R1 = NBLK * CPB1 * P
    R2 = NBLK * CPB2 * P
    f32 = mybir.dt.float32
    dmm = {"f32": f32, "bf16": mybir.dt.bfloat16,
           "f16": mybir.dt.float16}[MM_DT]
    nc = bacc.Bacc("TRN2", target_bir_lowering=False, debug=False,
                   num_devices=NC)
    E_full = nc.dram_tensor("E_full", [N_EDGES, HID], dmm,
                            kind="ExternalInput").ap()
    doh = {"f8": mybir.dt.float8e4, "f16": mybir.dt.float16}[OH_DT]
    E_p1 = nc.dram_tensor("E_p1", [R1, HID], dmm, kind="ExternalInput").ap()
    S = nc.dram_tensor("S", [P, R1], doh, kind="ExternalInput").ap()
    rev_ids = nc.dram_tensor("rev_ids", [P, NBLK * CPB2], mybir.dt.int32,
                             kind="ExternalInput").ap()
    R = nc.dram_tensor("R", [P, R2], doh, kind="ExternalInput").ap()
    Wt = nc.dram_tensor("Wt", [2, P, HID], dmm, kind="ExternalInput").ap()
    bias = nc.dram_tensor("bias", [P, HID], f32, kind="ExternalInput").ap()
    ident = nc.dram_tensor("ident", [P, P], dmm, kind="ExternalInput").ap()
    out = nc.dram_tensor("out", [R2, HID], dmm, kind="ExternalOutput").ap()

    with tile.TileContext(nc) as tc:
        with ExitStack() as ctx:
            const = ctx.enter_context(tc.tile_pool(name="const", bufs=1))
            sb = ctx.enter_context(tc.tile_pool(name="sb", bufs=SB_BUFS))
            mvp = ctx.enter_context(tc.tile_pool(name="mv", bufs=1))
            ps_mv = ctx.enter_context(
                tc.tile_pool(name="ps_mv", bufs=PS_BUFS[0], space="PSUM"))
            ps_pv = ctx.enter_context(
                tc.tile_pool(name="ps_pv", bufs=PS_BUFS[1], space="PSUM"))
            ps_tr = ctx.enter_context(
                tc.tile_pool(name="ps_tr", bufs=PS_BUFS[2], space="PSUM"))
            ps_out = ctx.enter_context(
                tc.tile_pool(name="ps_out", bufs=PS_BUFS[3], space="PSUM"))

            # constants
            wt0 = const.tile([P, HID], dmm)
            nc.sync.dma_start(out=wt0[:], in_=Wt[0])
            wt1 = const.tile([P, HID], dmm)
            nc.sync.dma_start(out=wt1[:], in_=Wt[1])
            bias_t = const.tile([P, HID], f32)
            nc.sync.dma_start(out=bias_t[:], in_=bias[:])
            ident_t = const.tile([P, P], dmm)
            nc.sync.dma_start(out=ident_t[:], in_=ident[:])
            if BIAS_PE:
                ones_t = const.tile([1, P], dmm)
                nc.gpsimd.memset(ones_t[:], 1.0)
                brow_t = const.tile([1, HID], dmm)
                nc.gpsimd.dma_start(out=brow_t[:], in_=bias[0:1, :])
            else:
                ones_t = brow_t = None
            rev_t = const.tile([P, NBLK * CPB2], mybir.dt.int32)
            nc.sync.dma_start(out=rev_t[:], in_=rev_ids[:])

            mv_all = mvp.tile([P, NBLK * HID], dmm)  # resident M_v

            for _rep in range(reps):
                _emit_body(nc, tc, locals(), CPB1, CPB2)
    nc.compile()
    return nc


def _emit_body(nc, tc, env, CPB1, CPB2):
    f32 = mybir.dt.float32
    dmm = {"f32": f32, "bf16": mybir.dt.bfloat16,
           "f16": mybir.dt.float16}[MM_DT]
    doh = {"f8": mybir.dt.float8e4, "f16": mybir.dt.float16}[OH_DT]
    sb, mv_all = env["sb"], env["mv_all"]
    ps_mv, ps_pv, ps_tr, ps_out = (env["ps_mv"], env["ps_pv"], env["ps_tr"],
                                   env["ps_out"])
    E_p1, E_full, out = env["E_p1"], env["E_full"], env["out"]
    S, R = env["S"], env["R"]
    rev_t = env["rev_t"]
    ident_t = env["ident_t"]
    wt0, wt1, bias_t = env["wt0"], env["wt1"], env["bias_t"]
    ones_t, brow_t = env.get("ones_t"), env.get("brow_t")
    for bb in range(NBLK):
        if True:
            # ---------------- phase 1: segment sum ----------------
            if not SKIP_P1:
                h_blk = sb.tile([P, CPB1 * HID], dmm, tag="h_blk")
                base = bb * CPB1 * P
                nc.sync.dma_start(
                    out=h_blk[:].rearrange("p (j d) -> p j d", j=CPB1),
                    in_=E_p1[base:base + CPB1 * P, :].rearrange(
                        "(j p) d -> p j d", p=P))
                s_blk = sb.tile([P, CPB1 * P], doh, tag="s_blk")
                nc.sync.dma_start(
                    out=s_blk[:],
                    in_=S[:, bb * CPB1 * P:(bb + 1) * CPB1 * P])
                mv_ps = ps_mv.tile([P, HID], f32, space="PSUM")
                for j in range(CPB1):
                    nc.tensor.matmul(
                        out=mv_ps[:], lhsT=s_blk[:, j * P:(j + 1) * P],
                        rhs=h_blk[:, j * HID:(j + 1) * HID],
                        start=(j == 0), stop=(j == CPB1 - 1))
                nc.vector.tensor_copy(
                    out=mv_all[:, bb * HID:(bb + 1) * HID], in_=mv_ps[:])

            # ------------- phase 2: gather-subtract-linear (same block) -------------
            if True:
                er_blk = sb.tile([P, CPB2 * HID], dmm, tag="er_blk")
                if not SKIP_REV:
                    for j0 in range(0, CPB2, GB):
                        g = min(GB, CPB2 - j0)
                        col = bb * CPB2 + j0
                        if g == 1:
                            o_ap = er_blk[:, j0 * HID:(j0 + 1) * HID]
                        else:
                            o_ap = er_blk[:, j0 * HID:(j0 + g) * HID
                                          ].rearrange("p (j d) -> p j d", j=g)
                        nc.gpsimd.indirect_dma_start(
                            out=o_ap,
                            out_offset=None,
                            in_=E_full[:],
                            in_offset=bass.IndirectOffsetOnAxis(
                                ap=rev_t[:, col:col + g], axis=0))
                else:
                    nc.gpsimd.memset(er_blk[:], 0.0)
                r_blk = sb.tile([P, CPB2 * P], doh, tag="r_blk")
                base = bb * CPB2 * P
                nc.sync.dma_start(out=r_blk[:],
                                  in_=R[:, base:base + CPB2 * P])
                out_blk = sb.tile([P, CPB2 * HID], dmm, tag="out_blk")
                for j in range(CPB2):
                    pv_ps = ps_pv.tile([P, HID], f32, space="PSUM")
                    nc.tensor.matmul(
                        out=pv_ps[:], lhsT=r_blk[:, j * P:(j + 1) * P],
                        rhs=mv_all[:, bb * HID:(bb + 1) * HID],
                        start=True, stop=True)
                    muv = sb.tile([P, HID], dmm, tag="muv")
                    nc.vector.tensor_tensor(
                        out=muv[:], in0=pv_ps[:],
                        in1=er_blk[:, j * HID:(j + 1) * HID],
                        op=mybir.AluOpType.subtract)
                    if not SKIP_LIN:
                        tr_ps = ps_tr.tile([P, HID], dmm, space="PSUM")
                        nc.tensor.transpose(tr_ps[:, 0:P], muv[:, 0:P],
                                            ident_t[:])
                        nc.tensor.transpose(tr_ps[:, P:HID], muv[:, P:HID],
                                            ident_t[:])
                        t_sb = sb.tile([P, HID], dmm, tag="t_sb")
                        if TCOPY_ACT:
                            nc.scalar.activation(
                                t_sb[:], tr_ps[:],
                                mybir.ActivationFunctionType.Copy)
                        else:
                            nc.vector.tensor_copy(out=t_sb[:], in_=tr_ps[:])
                        out_ps = ps_out.tile([P, HID], f32, space="PSUM")
                        nc.tensor.matmul(out=out_ps[:], lhsT=t_sb[:, 0:P],
                                         rhs=wt0[:], start=True, stop=False)
                        nc.tensor.matmul(out=out_ps[:], lhsT=t_sb[:, P:HID],
                                         rhs=wt1[:], start=False,
                                         stop=not BIAS_PE)
                        if BIAS_PE:
                            nc.tensor.matmul(out=out_ps[:], lhsT=ones_t[:],
                                             rhs=brow_t[:], start=False,
                                             stop=True)
                            nc.scalar.activation(
                                out_blk[:, j * HID:(j + 1) * HID], out_ps[:],
                                mybir.ActivationFunctionType.Copy)
                        else:
                            nc.vector.tensor_tensor(
                                out=out_blk[:, j * HID:(j + 1) * HID],
                                in0=out_ps[:], in1=bias_t[:],
                                op=mybir.AluOpType.add)
                    else:
                        nc.vector.tensor_copy(
                            out=out_blk[:, j * HID:(j + 1) * HID], in_=muv[:])
                nc.sync.dma_start(
                    out=out[base:base + CPB2 * P, :].rearrange(
                        "(j p) d -> p j d", p=P),
                    in_=out_blk[:].rearrange("p (j d) -> p j d", j=CPB2))


def assemble(results, meta):
    out_full = np.empty((N_EDGES, HID), np.float32)
    for c in range(NC):
        mc = meta["metas"][c]
        out_full[mc["e2"]] = np.asarray(
            results[c]["out"])[mc["row2"]].astype(np.float32)
    return out_full


def kernel(E, edge_index, rev_index, W, b):
    in_maps, meta = prepare(E, edge_index, rev_index, W, b)
    nc = build_program(meta["CPB1"], meta["CPB2"])
    res = run_bass_kernel_spmd(nc, in_maps, list(range(NC)))
    return assemble(res.results, meta)

